# revision 1
# baseline (speedup 1.0000x reference)
"""Chamfer distance loss kernel for Trainium2 (8 NeuronCores).

Problem: template/source (4, 8192, 3) f32. For each batch b:
  d[n,m] = |t_n|^2 - 2 t_n.s_m + |s_m|^2
  loss_b = mean_n min_m d + mean_m min_n d ; output = mean_b loss_b (scalar).

Strategy: z-sorted banding. Both point sets are sorted by z on the host
(per batch). A 128-row template tile then only needs a contiguous band
of ~1024 z-sorted source columns: nearest neighbors are z-close for all
but a handful of radial outliers. This cuts the distance-matrix volume
8x (32 tiles x 1024 cols per core instead of 32 x 8192).

The banding is made EXACT by a host-side patch: a banded row-min can
only be wrong if the true nearest neighbor lies outside the band, which
implies d >= (z-gap to the band edge)^2. Rows/cols whose banded min
exceeds that bound are recomputed exactly in numpy (a few hundred per
batch, milliseconds). Means are permutation-invariant so the sort needs
no undoing.

Sharding: core c = (batch c//2, sorted-template-half c%2); each core's
source input is a pre-shifted 5120-rank window of the sorted source
(rank range [4096h-448, 4096h+4672)) padded with far-away sentinel
points, which keeps one SPMD program valid for both halves: tile i
always scans virtual columns [128i, 128i+1024).

Per core, each [128 x 512] distance tile is produced in PSUM by ONE
augmented matmul: d = |t|^2 - 2 t.s + |s|^2 as a K=24 contraction of
bf16 3-way value splits (fp32-grade distances at 1 cycle/row). ScalarE
casts each 1024-col PSUM group to fp16 once; VectorE does the row-min
reduce and the column-min accumulate in fp16 2x packed mode. Column
minima ship to HBM in 1024-col chunks as soon as their last touching
tile has passed, overlapping output DMA with compute.
"""
import os
import sys

sys.path.insert(0, "/opt/trn_rl_repo")

from contextlib import ExitStack

import numpy as np

import concourse.bass as bass
import concourse.tile as tile
from concourse import mybir
from concourse.bass_utils import run_bass_kernel_spmd

# ---------------------------------------------------------------------------
# The walrus build in this container rejects instructions carrying more than
# one sync-wait command. After Tile scheduling, split any multi-wait
# instruction: keep the first wait on it and hoist the rest onto standalone
# EventSemaphore instructions inserted just before it (same engine, so
# per-engine program order makes the waits execute first).
import bass_rust as _br


def split_multi_waits(nc):
    n_new = 0
    for fn in nc.m.functions:
        for blk in fn.blocks:
            insts = list(blk.instructions)
            out = []
            changed = False
            for inst in insts:
                si = inst.sync_info
                waits = list(si.on_wait) if si is not None and si.on_wait else []
                if len(waits) > 1:
                    for w in waits[:-1]:
                        ev = _br.InstEventSemaphore(
                            name=f"I-waitsplit-{n_new}", ins=[], outs=[]
                        )
                        n_new += 1
                        ev.engine = inst.engine
                        ev.sync_info = _br.SyncInfo(on_wait=[w], on_update=[])
                        out.append(ev)
                    si.on_wait = [waits[-1]]
                    changed = True
                out.append(inst)
            if changed:
                blk.instructions = out
# ---------------------------------------------------------------------------

import ml_dtypes

F32 = mybir.dt.float32
F16 = mybir.dt.float16
BF16 = mybir.dt.bfloat16
MIN = mybir.AluOpType.min
BF16NP = ml_dtypes.bfloat16

B, N, M, D = 4, 8192, 8192, 3
HALF = N // 2       # template rows per core
NCORES = 8
K = 24              # dekker-split contraction depth
W = 192             # band half-width in source ranks
BAND = 512          # columns per tile band (= 128 + 2*W)
VCOLS = HALF + 2 * W + 128  # virtual source columns per core = 6144
ROW_TILES = HALF // 128     # 32
BIG = 60000.0       # fp16-safe "+inf" for min accumulators
SENT_SQ = 30000.0   # sentinel |s|^2: d at padded columns never wins a min


# row tiles per PSUM group: small lead-in groups fill the pipeline sooner,
# groups of 3 keep the matmul burst (~1.3us) under the cast (~1.6us) so the
# ScalarE pipeline never stalls, and a small tail group shrinks the final
# exposed DMA
GROUPS = [1, 1, 2] + [3] * 8 + [2, 2]
assert sum(GROUPS) == ROW_TILES


def build_program(split_waits=True):
    nc = bass.Bass("TRN2", target_bir_lowering=False, debug=False)
    lhsT = nc.dram_tensor("lhsT_aug", [K, HALF], BF16, kind="ExternalInput").ap()
    rhs = nc.dram_tensor("rhs_aug", [K, VCOLS], BF16, kind="ExternalInput").ap()
    o_rm = nc.dram_tensor(
        "out_rowmin", [128, ROW_TILES], F16, kind="ExternalOutput"
    ).ap()
    # the raw banded fp16 distance blocks; the host derives column minima
    # from them (each column is covered by only ~4 tiles)
    o_ct = nc.dram_tensor(
        "out_cst", [128, ROW_TILES * 512], F16, kind="ExternalOutput"
    ).ap()

    with tile.TileContext(nc) as tc, ExitStack() as ctx:
        consts = ctx.enter_context(tc.tile_pool(name="consts", bufs=1))
        psum_pool = ctx.enter_context(tc.tile_pool(name="psum", bufs=2, space="PSUM"))
        cast_pool = ctx.enter_context(tc.tile_pool(name="cast", bufs=3))
        rfold_pool = ctx.enter_context(tc.tile_pool(name="rfold", bufs=2))
        accs = ctx.enter_context(tc.tile_pool(name="accs", bufs=1))

        # warm the ACT function-table (its ~2.7us load overlaps input DMA)
        warm = consts.tile([1, 16], F16)
        nc.vector.memset(warm[:], 0.0)
        nc.scalar.copy(warm[:], warm[:])

        lhsT_sb = consts.tile([K, HALF], BF16)
        rhs_sb = consts.tile([K, VCOLS], BF16)
        # first groups' operands land first so matmuls start early
        nc.sync.dma_start(lhsT_sb[:, 0:512], lhsT[:, 0:512])
        nc.sync.dma_start(rhs_sb[:, 0:1024], rhs[:, 0:1024])
        nc.sync.dma_start(rhs_sb[:, 1024:VCOLS], rhs[:, 1024:VCOLS])
        nc.sync.dma_start(lhsT_sb[:, 512:HALF], lhsT[:, 512:HALF])

        rowminb = accs.tile([128, ROW_TILES], F16)

        i0 = 0
        for tpg in GROUPS:
            ps = psum_pool.tile([128, tpg * 512], F32)
            for t in range(tpg):
                i = i0 + t
                nc.tensor.matmul(
                    ps[:, t * 512:(t + 1) * 512],
                    lhsT_sb[:, i * 128:(i + 1) * 128],
                    rhs_sb[:, i * 128:i * 128 + BAND],
                    start=True, stop=True,
                )
            cst = cast_pool.tile([128, tpg, 512], F16)
            nc.scalar.copy(cst[:], ps[:])
            # per-tile row minima: fold 512->128 with 2x-mode tensor_tensor
            # ops, then one batched 1x reduce emits tpg columns at once
            rf = rfold_pool.tile([128, tpg, 256], F16)
            nc.vector.tensor_tensor(
                rf[:], cst[:, :, 0:256], cst[:, :, 256:512], op=MIN)
            nc.vector.tensor_tensor(
                rf[:, :, 0:128], rf[:, :, 0:128], rf[:, :, 128:256], op=MIN)
            nc.vector.tensor_reduce(
                rowminb[:, i0:i0 + tpg], rf[:, :, 0:128],
                axis=mybir.AxisListType.X, op=MIN,
            )
            nc.sync.dma_start(
                o_ct[:, i0 * 512:(i0 + tpg) * 512], cst[:])
            i0 += tpg
        nc.sync.dma_start(o_rm, rowminb[:])
    if split_waits:
        split_multi_waits(nc)  # CoreSim can't model the injected waits
    return nc


_program_cache = {}


def _get_program():
    if "p" not in _program_cache:
        _program_cache["p"] = build_program()
    return _program_cache["p"]


def enable_profiling():
    """Wire up the NTFF profiling hook (the image's antenv lacks
    antenv.axon_hooks) and neuter the credential-requiring artifact upload.
    Needed only when tracing (BASS_TRACE=1); harmless otherwise."""
    import types
    import antenv
    import concourse.bass_utils as _bu

    if "antenv.axon_hooks" not in sys.modules:
        hooks = types.ModuleType("antenv.axon_hooks")
        hooks._h = None
        hooks.set_axon_ntff_profile_hook = lambda h: setattr(hooks, "_h", h)
        hooks.get_axon_ntff_profile_hook = lambda: hooks._h
        sys.modules["antenv.axon_hooks"] = hooks
        antenv.axon_hooks = hooks
        try:
            from trn_agent_boot.trn_boot import _ntff_profile_via_ctypes

            hooks.set_axon_ntff_profile_hook(
                _ntff_profile_via_ctypes("/opt/axon/libaxon_pjrt.so")
            )
        except Exception:
            pass
    _bu.upload_artifacts = lambda tmpdir: f"local:{tmpdir}"


if os.environ.get("BASS_TRACE"):
    try:
        enable_profiling()
    except Exception:
        pass


def _split3(x):
    x1 = x.astype(BF16NP)
    r = x - x1.astype(np.float32)
    x2 = r.astype(BF16NP)
    x3 = (r - x2.astype(np.float32)).astype(BF16NP)
    return x1, x2, x3


def _aug_dekker(t, s, s_sq):
    """K=24 bf16 3-way-split augmentation. Each fp32 value a = a1+a2+a3 in
    bf16 parts; products kept to O(2^-27): a1b1, a1b2, a2b1, a1b3, a3b1,
    a2b2. PE cost is free-dim cycles only, so K=24 runs as fast as K=5."""
    rows, cols = t.shape[0], s.shape[0]
    t1, t2, t3 = _split3(t)
    s1, s2, s3 = _split3(-2.0 * s)
    n1, n2, n3 = _split3((t * t).sum(axis=1))
    m1, m2, m3 = _split3(s_sq)
    one = np.ones((), BF16NP)
    lhsT = np.empty((24, rows), BF16NP)
    for j, part in enumerate((t1, t1, t2, t1, t3, t2)):
        lhsT[3 * j:3 * j + 3] = part.T
    lhsT[18] = n1
    lhsT[19] = n2
    lhsT[20] = n3
    lhsT[21:24] = one
    rhs = np.empty((24, cols), BF16NP)
    for j, part in enumerate((s1, s2, s1, s3, s1, s2)):
        rhs[3 * j:3 * j + 3] = part.T
    rhs[18:21] = one
    rhs[21] = m1
    rhs[22] = m2
    rhs[23] = m3
    return lhsT, rhs


def _prep(template, source):
    """Sort per batch by z; build per-core shifted+padded source windows."""
    template = np.asarray(template, dtype=np.float32)
    source = np.asarray(source, dtype=np.float32)
    tb_all, sb_all, in_maps = [], [], []
    for b in range(B):
        tb = template[b][np.argsort(template[b][:, 2], kind="stable")]
        sb = source[b][np.argsort(source[b][:, 2], kind="stable")]
        tb_all.append(tb)
        sb_all.append(sb)
    for c in range(NCORES):
        b, h = c // 2, c % 2
        tb, sb = tb_all[b], sb_all[b]
        t = tb[h * HALF:(h + 1) * HALF]
        lo = HALF * h - W
        idx = np.arange(lo, lo + VCOLS)
        valid = (idx >= 0) & (idx < M)
        s = np.where(valid[:, None], sb[np.clip(idx, 0, M - 1)], 0.0)
        s_sq = np.where(valid, (s * s).sum(axis=1), SENT_SQ).astype(np.float32)
        lhsT, rhs = _aug_dekker(t, s.astype(np.float32), s_sq)
        in_maps.append(
            {"lhsT_aug": np.ascontiguousarray(lhsT),
             "rhs_aug": np.ascontiguousarray(rhs)}
        )
    return tb_all, sb_all, in_maps


last_results = None  # BassKernelResults of the most recent kernel() call


def kernel(template, source):
    global last_results
    nc = _get_program()
    tb_all, sb_all, in_maps = _prep(template, source)
    res = run_bass_kernel_spmd(nc, in_maps, list(range(NCORES)))
    last_results = res

    per_batch = np.zeros(B, dtype=np.float64)
    for b in range(B):
        tb = tb_all[b].astype(np.float64)
        sb = sb_all[b].astype(np.float64)
        ztb, zsb = tb[:, 2], sb[:, 2]

        rowmin = np.empty(N, dtype=np.float64)
        colmin = np.full(M, np.inf)
        # virtual-column index per (tile, band position)
        idxm = 128 * np.arange(ROW_TILES)[:, None] + np.arange(BAND)[None, :]
        for h in range(2):
            r = res.results[2 * b + h]
            rm = r["out_rowmin"].astype(np.float64)  # [128, 32]
            # column j of rm = per-row minima of tile j (rows = partitions)
            rowmin[h * HALF:(h + 1) * HALF] = rm.T.reshape(-1)
            # [128, 32*512] -> per-tile blocks -> min over partitions
            ct = r["out_cst"].astype(np.float64)
            tiles = ct.reshape(128, ROW_TILES, 512).transpose(1, 0, 2)
            colpart = tiles.min(axis=1)
            lo = HALF * h - W
            rank = idxm + lo
            valid = (rank >= 0) & (rank < M)
            np.minimum.at(colmin, np.clip(rank, 0, M - 1).ravel(),
                          np.where(valid, colpart, np.inf).ravel())

        # --- exactness patch: rows whose banded min could be beaten by an
        # out-of-band source point (d_outside >= z_gap^2) ---
        g = np.arange(N) // 128
        band_lo = HALF * (g // ROW_TILES) - W + 128 * (g % ROW_TILES)
        band_hi = band_lo + BAND
        gap_lo = np.where(band_lo > 0,
                          np.abs(ztb - zsb[np.clip(band_lo - 1, 0, M - 1)]),
                          np.inf)
        gap_hi = np.where(band_hi < M,
                          np.abs(zsb[np.clip(band_hi, 0, M - 1)] - ztb),
                          np.inf)
        gap2 = np.minimum(gap_lo, gap_hi) ** 2
        flag_r = np.where(rowmin > gap2 * 0.95 - 1e-4)[0]
        if len(flag_r):
            d = ((tb[flag_r][:, None, :] - sb[None, :, :]) ** 2).sum(-1)
            rowmin[flag_r] = d.min(axis=1)

        # --- exactness patch: columns (symmetric) ---
        tile_lo = HALF * (np.arange(2 * ROW_TILES) // ROW_TILES) - W \
            + 128 * (np.arange(2 * ROW_TILES) % ROW_TILES)
        v = np.arange(M)
        cov = (v[None, :] >= tile_lo[:, None]) & \
              (v[None, :] < tile_lo[:, None] + BAND)  # [64, M]
        n_lo = np.argmax(cov, axis=0) * 128
        n_hi = (len(cov) - 1 - np.argmax(cov[::-1], axis=0)) * 128 + 128
        zg_lo = np.where(n_lo > 0,
                         np.abs(zsb - ztb[np.clip(n_lo - 1, 0, N - 1)]),
                         np.inf)
        zg_hi = np.where(n_hi < N,
                         np.abs(ztb[np.clip(n_hi, 0, N - 1)] - zsb),
                         np.inf)
        gap2c = np.minimum(zg_lo, zg_hi) ** 2
        flag_c = np.where(colmin > gap2c * 0.95 - 1e-4)[0]
        if len(flag_c):
            d = ((tb[None, :, :] - sb[flag_c][:, None, :]) ** 2).sum(-1)
            colmin[flag_c] = d.min(axis=1)

        per_batch[b] = rowmin.mean() + colmin.mean()
    return np.float32(per_batch.mean())



# revision 13
# speedup vs baseline: 1.0181x; 1.0181x over previous
"""Chamfer distance loss kernel for Trainium2 (8 NeuronCores).

Problem: template/source (4, 8192, 3) f32. For each batch b:
  d[n,m] = |t_n|^2 - 2 t_n.s_m + |s_m|^2
  loss_b = mean_n min_m d + mean_m min_n d ; output = mean_b loss_b (scalar).

Strategy: z-sorted banding, direction-sharded. Both point sets are
sorted by z on the host (per batch). Core c = (batch c//2, direction
c%2): direction 0 computes banded row-mins for all 8192 template rows
against a 256-wide z-band of source columns; direction 1 swaps roles
(source rows vs template band), which IS the column-min of the distance
matrix. Both directions are plain free-axis row reductions on device --
no partition-axis min, no distance-block shipping (the old kernel
shipped 4.2MB/core of banded distances for host-side column minima).

The banding is made EXACT by a host-side patch: a banded row-min can
only be wrong if the true nearest neighbor lies outside the band, which
implies d >= (z-gap to the band edge)^2. Rows whose banded min exceeds
that bound are recomputed exactly in numpy (W=64 flags 5-20% of rows;
a few hundred ms). Means are permutation-invariant so the sort needs
no undoing.

Per core, each [128 x 256] distance tile is ONE augmented matmul
(K=24 bf16 3-way value splits, fp32-grade distances). 64 tiles in 8
super-groups of 8 share a 4-bank PSUM tile. Row-min reduce is split
across engines to keep every engine under the Tensor engine's time:
5 groups go Scalar-cast (f32->f16) -> Vector f16 2x folds; 3 groups go
Vector fold directly from PSUM f32; fold3 + final 32->1 reduce run on
GpSimd. A short warmup matmul chain keeps the PE p-state ramp (0.65 ->
1.2 -> 2.4 GHz after 3us continuous) moving during input DMA, and
inputs are DMAed in 16 column-chunks across queues so tile 0's operands
land in ~1us.
"""
import os
import sys

sys.path.insert(0, "/opt/trn_rl_repo")

from contextlib import ExitStack

import numpy as np

import concourse.bass as bass
import concourse.tile as tile
from concourse import mybir
from concourse.bass_utils import run_bass_kernel_spmd

# ---------------------------------------------------------------------------
# The walrus build in this container rejects instructions carrying more than
# one sync-wait command. After Tile scheduling, split any multi-wait
# instruction: keep the first wait on it and hoist the rest onto standalone
# EventSemaphore instructions inserted just before it (same engine, so
# per-engine program order makes the waits execute first).
import bass_rust as _br


def split_multi_waits(nc):
    n_new = 0
    for fn in nc.m.functions:
        for blk in fn.blocks:
            insts = list(blk.instructions)
            out = []
            changed = False
            for inst in insts:
                si = inst.sync_info
                waits = list(si.on_wait) if si is not None and si.on_wait else []
                if len(waits) > 1:
                    for w in waits[:-1]:
                        ev = _br.InstEventSemaphore(
                            name=f"I-waitsplit-{n_new}", ins=[], outs=[]
                        )
                        n_new += 1
                        ev.engine = inst.engine
                        ev.sync_info = _br.SyncInfo(on_wait=[w], on_update=[])
                        out.append(ev)
                    si.on_wait = [waits[-1]]
                    changed = True
                out.append(inst)
            if changed:
                blk.instructions = out
# ---------------------------------------------------------------------------

import ml_dtypes

F32 = mybir.dt.float32
F16 = mybir.dt.float16
BF16 = mybir.dt.bfloat16
MIN = mybir.AluOpType.min
BF16NP = ml_dtypes.bfloat16

B, N, M, D = 4, 8192, 8192, 3
NCORES = 8
K = 24              # dekker-split contraction depth
W = 64              # band half-width in source ranks
BAND = 256          # columns per tile band (= 128 + 2*W)
VCOLS = N + 2 * W   # virtual band columns per core = 8320
ROW_TILES = N // 128        # 64
SENT_SQ = 30000.0   # sentinel |s|^2: d at padded columns never wins a min

G = 8               # tiles per super-group
NGROUPS = ROW_TILES // G    # 8
# which super-groups cast PSUM->f16 on Scalar (the rest cast on Vector);
# chosen so Scalar ~13us and Vector ~11us stay at/under the PE time
S_GROUPS = {1, 2, 3, 4, 5, 6, 7}


def build_program(split_waits=True):
    nc = bass.Bass("TRN2", target_bir_lowering=False, debug=False)
    lhsT = nc.dram_tensor("lhsT_aug", [K, N], BF16, kind="ExternalInput").ap()
    rhs = nc.dram_tensor("rhs_aug", [K, VCOLS], BF16, kind="ExternalInput").ap()
    # [part, group, bank, half]: tile i = 8*(i//8) + 4*half + bank
    o_rm = nc.dram_tensor(
        "out_rowmin", [128, NGROUPS, 4, 2], F16, kind="ExternalOutput"
    ).ap()

    with tile.TileContext(nc) as tc, ExitStack() as ctx:
        consts = ctx.enter_context(tc.tile_pool(name="consts", bufs=1))
        psum_pool = ctx.enter_context(tc.tile_pool(name="psum", bufs=2, space="PSUM"))
        cast_pool = ctx.enter_context(tc.tile_pool(name="cast", bufs=2))
        m1_pool = ctx.enter_context(tc.tile_pool(name="m1", bufs=2))
        m2_pool = ctx.enter_context(tc.tile_pool(name="m2", bufs=2))
        m3_pool = ctx.enter_context(tc.tile_pool(name="m3", bufs=2))
        m4_pool = ctx.enter_context(tc.tile_pool(name="m4", bufs=2))
        accs = ctx.enter_context(tc.tile_pool(name="accs", bufs=1))

        # warm the ACT function-table (its ~1.3us load overlaps input DMA)
        warm = consts.tile([1, 16], F16)
        nc.vector.memset(warm[:], 0.0)
        nc.scalar.copy(warm[:], warm[:])

        lhsT_sb = consts.tile([K, N], BF16)
        rhs_sb = consts.tile([K, VCOLS], BF16)
        # 16 column-chunks across DMA queues, earliest tiles' operands first
        LC = N // 8       # 1024 lhsT cols per chunk
        RC = VCOLS // 8   # 1040 rhs cols per chunk
        for c in range(8):
            nc.sync.dma_start(rhs_sb[:, c * RC:(c + 1) * RC],
                              rhs[:, c * RC:(c + 1) * RC])
            nc.sync.dma_start(lhsT_sb[:, c * LC:(c + 1) * LC],
                              lhsT[:, c * LC:(c + 1) * LC])

        # PE p-state warmup: keep the tensor engine busy while inputs land
        warm_l = consts.tile([1, 128], BF16)
        warm_r = consts.tile([1, 512], BF16)
        nc.vector.memset(warm_l[:], 0.0)
        nc.vector.memset(warm_r[:], 0.0)
        # shares the "ps" tag (pool slots are keyed by tile name) so the
        # warmup matmuls borrow one of the two group PSUM buffers
        wps = psum_pool.tile([128, 4, 2, 256], F32, name="ps")
        for _ in range(4):
            nc.tensor.matmul(wps[:, 0:1, :, :], warm_l[:], warm_r[:],
                             start=True, stop=True)

        rowminb = accs.tile([128, NGROUPS, 4, 2], F16)

        for g in range(NGROUPS):
            # two 256-col tiles share each 2KB PSUM bank row (tile t lives
            # at [bank t%4, half t//4]); a walrus rule allows only ONE
            # PSUM input per instruction, so every group is first cast
            # f32->f16 by a single-input copy, then min-folded in f16
            ps = psum_pool.tile([128, 4, 2, 256], F32)
            for t in range(G):
                i = g * G + t
                b, h = t % 4, t // 4
                nc.tensor.matmul(
                    ps[:, b:b + 1, h:h + 1, :],
                    lhsT_sb[:, i * 128:(i + 1) * 128],
                    rhs_sb[:, i * 128:i * 128 + BAND],
                    start=True, stop=True,
                )
            cst = cast_pool.tile([128, 4, 2, 256], F16)
            if g in S_GROUPS:
                nc.scalar.copy(cst[:], ps[:])
            else:
                nc.vector.tensor_copy(cst[:], ps[:])
            m1 = m1_pool.tile([128, 4, 2, 128], F16)
            nc.vector.tensor_tensor(
                m1[:], cst[:, :, :, 0:128], cst[:, :, :, 128:256], op=MIN)
            m2 = m2_pool.tile([128, 4, 2, 64], F16)
            nc.vector.tensor_tensor(
                m2[:], m1[:, :, :, 0:64], m1[:, :, :, 64:128], op=MIN)
            m3 = m3_pool.tile([128, 4, 2, 32], F16)
            nc.vector.tensor_tensor(
                m3[:], m2[:, :, :, 0:32], m2[:, :, :, 32:64], op=MIN)
            m4 = m4_pool.tile([128, 4, 2, 16], F16)
            nc.vector.tensor_tensor(
                m4[:], m3[:, :, :, 0:16], m3[:, :, :, 16:32], op=MIN)
            nc.vector.tensor_reduce(
                rowminb[:, g, :, :], m4[:],
                axis=mybir.AxisListType.X, op=MIN,
            )
        nc.sync.dma_start(o_rm, rowminb[:])
    if split_waits:
        split_multi_waits(nc)  # CoreSim can't model the injected waits
    return nc


_program_cache = {}


def _get_program():
    if "p" not in _program_cache:
        _program_cache["p"] = build_program()
    return _program_cache["p"]


def enable_profiling():
    """Wire up the NTFF profiling hook (the image's antenv lacks
    antenv.axon_hooks) and neuter the credential-requiring artifact upload.
    Needed only when tracing (BASS_TRACE=1); harmless otherwise."""
    import types
    import antenv
    import concourse.bass_utils as _bu

    if "antenv.axon_hooks" not in sys.modules:
        hooks = types.ModuleType("antenv.axon_hooks")
        hooks._h = None
        hooks.set_axon_ntff_profile_hook = lambda h: setattr(hooks, "_h", h)
        hooks.get_axon_ntff_profile_hook = lambda: hooks._h
        sys.modules["antenv.axon_hooks"] = hooks
        antenv.axon_hooks = hooks
        try:
            from trn_agent_boot.trn_boot import _ntff_profile_via_ctypes

            hooks.set_axon_ntff_profile_hook(
                _ntff_profile_via_ctypes("/opt/axon/libaxon_pjrt.so")
            )
        except Exception:
            pass
    _bu.upload_artifacts = lambda tmpdir: f"local:{tmpdir}"


if os.environ.get("BASS_TRACE"):
    try:
        enable_profiling()
    except Exception:
        pass


def _split3(x):
    x1 = x.astype(BF16NP)
    r = x - x1.astype(np.float32)
    x2 = r.astype(BF16NP)
    x3 = (r - x2.astype(np.float32)).astype(BF16NP)
    return x1, x2, x3


def _aug_dekker(t, s, s_sq):
    """K=24 bf16 3-way-split augmentation. Each fp32 value a = a1+a2+a3 in
    bf16 parts; products kept to O(2^-27): a1b1, a1b2, a2b1, a1b3, a3b1,
    a2b2. PE cost is free-dim cycles only, so K=24 runs as fast as K=5."""
    rows, cols = t.shape[0], s.shape[0]
    t1, t2, t3 = _split3(t)
    s1, s2, s3 = _split3(-2.0 * s)
    n1, n2, n3 = _split3((t * t).sum(axis=1))
    m1, m2, m3 = _split3(s_sq)
    one = np.ones((), BF16NP)
    lhsT = np.empty((24, rows), BF16NP)
    for j, part in enumerate((t1, t1, t2, t1, t3, t2)):
        lhsT[3 * j:3 * j + 3] = part.T
    lhsT[18] = n1
    lhsT[19] = n2
    lhsT[20] = n3
    lhsT[21:24] = one
    rhs = np.empty((24, cols), BF16NP)
    for j, part in enumerate((s1, s2, s1, s3, s1, s2)):
        rhs[3 * j:3 * j + 3] = part.T
    rhs[18:21] = one
    rhs[21] = m1
    rhs[22] = m2
    rhs[23] = m3
    return lhsT, rhs


def _prep(template, source):
    """Sort per batch by z; per core (batch, direction) build the full
    8192-row lhsT and the W-shifted sentinel-padded band rhs."""
    template = np.asarray(template, dtype=np.float32)
    source = np.asarray(source, dtype=np.float32)
    tb_all, sb_all, in_maps = [], [], []
    for b in range(B):
        tb = template[b][np.argsort(template[b][:, 2], kind="stable")]
        sb = source[b][np.argsort(source[b][:, 2], kind="stable")]
        tb_all.append(tb)
        sb_all.append(sb)
    idx = np.arange(-W, N + W)
    valid = (idx >= 0) & (idx < N)
    cidx = np.clip(idx, 0, N - 1)
    for c in range(NCORES):
        b, d = c // 2, c % 2
        X = tb_all[b] if d == 0 else sb_all[b]   # rows
        Y = sb_all[b] if d == 0 else tb_all[b]   # band columns
        s = np.where(valid[:, None], Y[cidx], 0.0)
        s_sq = np.where(valid, (s * s).sum(axis=1), SENT_SQ).astype(np.float32)
        lhsT, rhs = _aug_dekker(X, s.astype(np.float32), s_sq)
        in_maps.append(
            {"lhsT_aug": np.ascontiguousarray(lhsT),
             "rhs_aug": np.ascontiguousarray(rhs)}
        )
    return tb_all, sb_all, in_maps


last_results = None  # BassKernelResults of the most recent kernel() call


def _patched_rowmin(res_core, X, Y):
    """Device banded row-mins for sorted rows X vs sorted cols Y, with the
    exactness patch: rows whose banded min could be beaten by an
    out-of-band point (d_outside >= z_gap^2) are recomputed in numpy."""
    rm = res_core["out_rowmin"].astype(np.float64)   # [128, 8, 4, 2]
    rm = rm.reshape(128, NGROUPS, 4, 2)
    # tile i = 8*(i//8) + 4*half + bank; row r = 128*i + partition
    rowmin = rm.transpose(1, 3, 2, 0).reshape(-1)
    zx, zy = X[:, 2], Y[:, 2]
    g = np.arange(N) // 128
    band_lo = 128 * g - W
    band_hi = band_lo + BAND
    gap_lo = np.where(band_lo > 0,
                      np.abs(zx - zy[np.clip(band_lo - 1, 0, N - 1)]),
                      np.inf)
    gap_hi = np.where(band_hi < N,
                      np.abs(zy[np.clip(band_hi, 0, N - 1)] - zx),
                      np.inf)
    gap2 = np.minimum(gap_lo, gap_hi) ** 2
    flag = np.where(rowmin > gap2 * 0.95 - 1e-4)[0]
    if len(flag):
        dd = ((X[flag][:, None, :] - Y[None, :, :]) ** 2).sum(-1)
        rowmin[flag] = dd.min(axis=1)
    return rowmin


def kernel(template, source):
    global last_results
    nc = _get_program()
    tb_all, sb_all, in_maps = _prep(template, source)
    res = run_bass_kernel_spmd(nc, in_maps, list(range(NCORES)))
    last_results = res

    per_batch = np.zeros(B, dtype=np.float64)
    for b in range(B):
        tb = tb_all[b].astype(np.float64)
        sb = sb_all[b].astype(np.float64)
        rmin = _patched_rowmin(res.results[2 * b + 0], tb, sb)
        cmin = _patched_rowmin(res.results[2 * b + 1], sb, tb)
        per_batch[b] = rmin.mean() + cmin.mean()
    return np.float32(per_batch.mean())
